# revision 58
# baseline (speedup 1.0000x reference)
"""Trainium2 Bass kernel for nn_BinaryLayer (logic-gate network).

Computes: out[b, o] = OR_t AND_a x_in[b, weights[o, t, a]]
where x_in = [const_true | (x != 0) | ~(x != 0)]  (width 1 + 2*784 = 1569),
plus an or-mask: an (o, t) gate whose 16 indices are all 0 is forced False.

Strategy (8 NeuronCores, tensor-parallel over OUT: 128 outs per core):
  The AND over 16 selected booleans equals (count of true inputs == 16),
  and the count is LINEAR in x:
      count[b, j] = bias[j] + sum_f A[f, j] * x[b, f],   j = 32*o_local + t
  with A[f, j] = (#refs to x_f) - (#refs to ~x_f)  in [-16, 16] and
  bias[j] = (#const-true refs) + (#~x refs)  (or -16 for or-masked gates).
  All values are small integers => EXACT in fp8e4 (e4m3) matmul with fp32
  PSUM accumulation.  Then:
      out[b, o] = max_t Relu(count[b, 32*o + t] - 15)  (0/1 exact)

  Per core: PE does 256 fp8 DoubleRow matmuls [K=256, M=128, N=512]
  (count tensor), the Activation engine applies Relu(count-15) PSUM->SBUF,
  DVE max-reduces over the 32 or-terms, all pipelined.  The bias is folded
  into an extra constant-one row of x^T (row 784) whose A row holds bias.
"""

import numpy as np

B, F = 1024, 784
OUT, OR_T, AND_T = 1024, 32, 16
N_CORES = 8
K = 1024            # padded contraction dim: 784 features + bias row + zeros
KC = K // 128       # 8 k-chunks of 128 partitions
KP = KC // 2        # 4 DoubleRow k-pairs
OL = OUT // N_CORES  # 128 output columns per core
JC = OL * OR_T      # 4096 gate columns per core (j = 32*o_local + t)
BIAS_ROW = F        # row 784 of x^T is the constant-one bias input

_cache = {}

# pairs (of two adjacent 128-batch chunks) handled by the DVE-direct path:
# these emit raw max-counts (host thresholds at 15.5); act-path pairs emit
# 0/1 gate maxima (host thresholds at 0.5)
DIRECT_PAIRS = frozenset({7, 15})


def _build(reps=1, hw_loop=False, stages="full", split_dma=False):
    import contextlib

    import concourse.mybir as mybir
    import concourse.tile as tile
    from concourse.bacc import Bacc

    f32 = mybir.dt.float32
    bf16 = mybir.dt.bfloat16
    f8 = mybir.dt.float8e4
    Alu = mybir.AluOpType
    ActF = mybir.ActivationFunctionType
    DR = mybir.MatmulPerfMode.DoubleRow

    nc = Bacc("TRN2", target_bir_lowering=False, debug=False, num_devices=N_CORES)
    # layouts chosen so every input DMA moves >=2KB-contiguous per partition
    xt_t = nc.dram_tensor("xt", [KP, 128, 2, B], f8, kind="ExternalInput")
    a_t = nc.dram_tensor("a", [KP, 4, 128, 2, JC // 4], f8, kind="ExternalInput")
    out_t = nc.dram_tensor("out", [B, OL], bf16, kind="ExternalOutput")

    with tile.TileContext(nc) as tc:
        with (
            tc.tile_pool(name="w", bufs=1) as wpool,
            tc.tile_pool(name="g", bufs=4) as gpool,
            tc.tile_pool(name="o", bufs=2) as opool,
            tc.tile_pool(name="psum", bufs=2, space="PSUM") as pp,
        ):
            neg15 = wpool.tile([128, 1], f32)
            nc.vector.memset(neg15[:], -15.0)
            xt_sb = wpool.tile([128, KC, B], f8)
            # a_sb[kp]: [128, q, pair, j-quarter]; quarter-granularity DMAs in
            # q-major order so arrival order matches consumption order; xt
            # interleaved with the q=0 slices so the first kp0 matmuls can
            # start after ~2 DMAs
            a_sb = [
                wpool.tile([128, 4, 2, JC // 4], f8, name=f"a_sb{kp}")
                for kp in range(KP)
            ]
            for kp in range(KP):
                nc.sync.dma_start(
                    xt_sb[:, 2 * kp : 2 * kp + 2, :], xt_t.ap()[kp]
                )
                nc.sync.dma_start(a_sb[kp][:, 0], a_t.ap()[kp][0])
                nc.sync.dma_start(a_sb[kp][:, 1], a_t.ap()[kp][1])
            for q in range(2, 4):
                for kp in range(KP):
                    nc.sync.dma_start(a_sb[kp][:, q], a_t.ap()[kp][q])

            if hw_loop:
                rep_ctx = lambda: tc.For_i(0, reps)  # noqa: E731
                n_py_reps = 1
            else:
                rep_ctx = contextlib.nullcontext
                n_py_reps = reps
            def post_pair(big, bc, h, base, pair_i):
                region = big[:, base : base + 2048]
                mxp = opool.tile([128, 2, 32], bf16, tag="mxp", bufs=4)
                if pair_i in DIRECT_PAIRS:
                    # DVE-direct: max-reduce fp32 straight from PSUM (only
                    # one PSUM input allowed per DVE op); split per slot so
                    # each psum slot releases as soon as its half is read;
                    # emits raw max-counts, host thresholds at 15.5
                    rv = region.rearrange("p (c o t) -> p c o t", c=2, t=OR_T)
                    for c in range(2):
                        nc.vector.tensor_reduce(
                            out=mxp[:, c],
                            in_=rv[:, c],
                            axis=mybir.AxisListType.X,
                            op=Alu.max,
                        )
                else:
                    gate = gpool.tile([128, 2048], bf16, tag="gate")
                    # split per 1024-col slot: each starts as soon as its slot's
                    # matmuls finish and releases that slot ~0.9us earlier
                    for c2 in range(2):
                        nc.scalar.activation(
                            out=gate[:, 1024 * c2 : 1024 * (c2 + 1)],
                            in_=region[:, 1024 * c2 : 1024 * (c2 + 1)],
                            func=ActF.Relu, bias=neg15[:],
                        )
                    # 2-level contiguous-half max tree (DVE 2x mode) + reduce
                    gv = gate[:].rearrange("p (c o t) -> p c o t", c=2, t=OR_T)
                    r1 = gpool.tile([128, 2, 32, 16], bf16, tag="r1")
                    nc.vector.tensor_tensor(
                        out=r1[:], in0=gv[:, :, :, 0:16],
                        in1=gv[:, :, :, 16:32], op=Alu.max,
                    )
                    r2 = gpool.tile([128, 2, 32, 8], bf16, tag="r2")
                    nc.vector.tensor_tensor(
                        out=r2[:], in0=r1[:, :, :, 0:8], in1=r1[:, :, :, 8:16],
                        op=Alu.max,
                    )
                    nc.vector.tensor_reduce(
                        out=mxp[:], in_=r2[:],
                        axis=mybir.AxisListType.X, op=Alu.max,
                    )
                # piece covers out rows [128bc, +128), cols [64h, 64h+64)
                nc.sync.dma_start(
                    out_t.ap()[
                        128 * bc : 128 * (bc + 1), 64 * h : 64 * (h + 1)
                    ],
                    mxp[:],
                )

            with rep_ctx():
              for _rep in range(n_py_reps):
                # one big PSUM tile managed as two 2-bank slot-pairs (subtile
                # deps give the PE lookahead).  A piece = (bc, q-half h):
                # 2048 cols of one batch chunk.  kp-outer inside the piece
                # so each lhsT load feeds 4 consecutive streams.
                big = pp.tile([128, 4096], f32, tag="big", bufs=1)
                pair_i = 0
                for h in range(2):
                    for bc in range(8):
                        base = 2048 * (pair_i % 2)
                        for kp in range(KP):
                            for dq in range(2):
                                q = 2 * h + dq
                                for n in range(2):
                                    nc.tensor.matmul(
                                        out=big[:, base + 1024 * dq + 512 * n :
                                                base + 1024 * dq + 512 * (n + 1)],
                                        lhsT=xt_sb[:, 2 * kp : 2 * kp + 2,
                                                   128 * bc : 128 * (bc + 1)],
                                        rhs=a_sb[kp][:, q, :,
                                                     512 * n : 512 * (n + 1)],
                                        start=(kp == 0),
                                        stop=(kp == KP - 1),
                                        perf_mode=DR,
                                    )
                        if stages.startswith("full"):
                            post_pair(big, bc, h, base, pair_i)
                        pair_i += 1
    nc.compile()
    return nc


def _host_inputs(x, weights):
    import ml_dtypes

    f8 = ml_dtypes.float8_e4m3
    ONE = int(np.array(1.0, dtype=f8).view(np.uint8))

    xb = np.asarray(x) != 0  # [B, F] bool
    xt = np.zeros((K, B), np.uint8)
    xt[:F][xb.T] = ONE
    xt[BIAS_ROW, :] = ONE
    # [K, B] -> [KP, 128, 2, B]; row r lives at (kp=r//256, p=r%128, pair=(r//128)%2)
    xt_in = np.ascontiguousarray(
        xt.reshape(KP, 2, 128, B).transpose(0, 2, 1, 3)
    ).view(f8)

    w = np.asarray(weights).astype(np.int64)       # [OUT, OR_T, AND_T]
    allzero = (w == 0).all(axis=-1)                # [OUT, OR_T]
    idx_all = w.reshape(OUT, OR_T * AND_T)
    lut = np.arange(-16, 17, dtype=np.float32).astype(f8).view(np.uint8)

    jj = np.repeat(np.arange(JC), AND_T)           # [JC*AND_T]
    a_maps = []
    for cc in range(N_CORES):
        idx = idx_all[OL * cc : OL * (cc + 1)].reshape(-1)  # [JC*AND_T]
        pos = (idx >= 1) & (idx <= F)
        neg = idx > F
        a_int = np.zeros(K * JC, np.int16)
        np.add.at(a_int, (idx[pos] - 1) * JC + jj[pos], 1)
        np.add.at(a_int, (idx[neg] - 1 - F) * JC + jj[neg], -1)
        a_int = a_int.reshape(K, JC)
        bias = (
            np.bincount(jj[idx == 0], minlength=JC)
            + np.bincount(jj[neg], minlength=JC)
        ).astype(np.int16)
        bias[allzero[OL * cc : OL * (cc + 1)].reshape(-1)] = -16
        a_int[BIAS_ROW, :] = bias
        a_u8 = lut[a_int + 16]                     # [K, JC] e4m3 bytes
        # [K, JC] -> [KP, 4, 128, 2, JC//4]
        a_maps.append(
            np.ascontiguousarray(
                a_u8.reshape(KP, 2, 128, 4, JC // 4).transpose(0, 3, 2, 1, 4)
            ).view(f8)
        )
    return xt_in, a_maps


def _assemble(results):
    import ml_dtypes

    out = np.empty((B, OUT), dtype=bool)
    for cc in range(N_CORES):
        arr = np.ascontiguousarray(results[cc]["out"])
        if arr.dtype != ml_dtypes.bfloat16:
            arr = arr.view(ml_dtypes.bfloat16)
        arr = arr.astype(np.float32)
        blk = np.empty((B, OL), dtype=bool)
        for pair_i in range(16):
            h, bc = divmod(pair_i, 8)
            thr = 15.5 if pair_i in DIRECT_PAIRS else 0.5
            rows = slice(128 * bc, 128 * (bc + 1))
            cols = slice(64 * h, 64 * (h + 1))
            blk[rows, cols] = arr[rows, cols] > thr
        out[:, OL * cc : OL * (cc + 1)] = blk
    return out


def kernel(x, weights):
    from concourse.bass_utils import run_bass_kernel_spmd

    if "nc" not in _cache:
        _cache["nc"] = _build(reps=1)
    nc = _cache["nc"]

    xt_in, a_maps = _host_inputs(x, weights)
    in_maps = [{"xt": xt_in, "a": a_maps[cc]} for cc in range(N_CORES)]
    # transient device/tunnel errors (NRT_EXEC_UNIT_UNRECOVERABLE): retry
    res = None
    for attempt in range(3):
        try:
            res = run_bass_kernel_spmd(
                nc, in_maps, core_ids=list(range(N_CORES))
            )
            break
        except Exception:
            if attempt == 2:
                raise
            import time

            time.sleep(5 * (attempt + 1))
    return _assemble(res.results)


# revision 60
# speedup vs baseline: 1.0569x; 1.0569x over previous
"""Trainium2 Bass kernel for nn_BinaryLayer (logic-gate network).

Computes: out[b, o] = OR_t AND_a x_in[b, weights[o, t, a]]
where x_in = [const_true | (x != 0) | ~(x != 0)]  (width 1 + 2*784 = 1569),
plus an or-mask: an (o, t) gate whose 16 indices are all 0 is forced False.

Strategy (8 NeuronCores, tensor-parallel over OUT: 128 outs per core):
  The AND over 16 selected booleans equals (count of true inputs == 16),
  and the count is LINEAR in x:
      count[b, j] = bias[j] + sum_f A[f, j] * x[b, f],   j = 32*o_local + t
  with A[f, j] = (#refs to x_f) - (#refs to ~x_f)  in [-16, 16] and
  bias[j] = (#const-true refs) + (#~x refs)  (or -16 for or-masked gates).
  All values are small integers => EXACT in fp8e4 (e4m3) matmul with fp32
  PSUM accumulation.  Then:
      out[b, o] = max_t Relu(count[b, 32*o + t] - 15)  (0/1 exact)

  Per core: PE does 256 fp8 DoubleRow matmuls [K=256, M=128, N=512]
  (count tensor), the Activation engine applies Relu(count-15) PSUM->SBUF,
  DVE max-reduces over the 32 or-terms, all pipelined.  The bias is folded
  into an extra constant-one row of x^T (row 784) whose A row holds bias.
"""

import numpy as np

B, F = 1024, 784
OUT, OR_T, AND_T = 1024, 32, 16
N_CORES = 8
K = 1024            # padded contraction dim: 784 features + bias row + zeros
KC = K // 128       # 8 k-chunks of 128 partitions
KP = KC // 2        # 4 DoubleRow k-pairs
OL = OUT // N_CORES  # 128 output columns per core
JC = OL * OR_T      # 4096 gate columns per core (j = 32*o_local + t)
BIAS_ROW = F        # row 784 of x^T is the constant-one bias input

_cache = {}

# pairs (of two adjacent 128-batch chunks) handled by the DVE-direct path:
# these emit raw max-counts (host thresholds at 15.5); act-path pairs emit
# 0/1 gate maxima (host thresholds at 0.5)
DIRECT_PAIRS = frozenset({7, 15})


def _build(reps=1, hw_loop=False, stages="full", split_dma=False):
    import contextlib

    import concourse.mybir as mybir
    import concourse.tile as tile
    from concourse.bacc import Bacc

    f32 = mybir.dt.float32
    bf16 = mybir.dt.bfloat16
    f8 = mybir.dt.float8e4
    Alu = mybir.AluOpType
    ActF = mybir.ActivationFunctionType
    DR = mybir.MatmulPerfMode.DoubleRow

    nc = Bacc("TRN2", target_bir_lowering=False, debug=False, num_devices=N_CORES)
    # layouts chosen so every input DMA moves >=2KB-contiguous per partition
    xt_t = nc.dram_tensor("xt", [KP, 128, 2, B], f8, kind="ExternalInput")
    a_t = nc.dram_tensor("a", [KP, 4, 128, 2, JC // 4], f8, kind="ExternalInput")
    out_t = nc.dram_tensor("out", [B, OL], bf16, kind="ExternalOutput")

    with tile.TileContext(nc) as tc:
        with (
            tc.tile_pool(name="w", bufs=1) as wpool,
            tc.tile_pool(name="g", bufs=4) as gpool,
            tc.tile_pool(name="o", bufs=2) as opool,
            tc.tile_pool(name="psum", bufs=2, space="PSUM") as pp,
        ):
            neg15 = wpool.tile([128, 1], f32)
            nc.vector.memset(neg15[:], -15.0)
            xt_sb = wpool.tile([128, KC, B], f8)
            # a_sb[kp]: [128, q, pair, j-quarter]; quarter-granularity DMAs in
            # q-major order so arrival order matches consumption order; xt
            # interleaved with the q=0 slices so the first kp0 matmuls can
            # start after ~2 DMAs
            a_sb = [
                wpool.tile([128, 4, 2, JC // 4], f8, name=f"a_sb{kp}")
                for kp in range(KP)
            ]
            for kp in range(KP):
                nc.sync.dma_start(
                    xt_sb[:, 2 * kp : 2 * kp + 2, :], xt_t.ap()[kp]
                )
                nc.sync.dma_start(a_sb[kp][:, 0], a_t.ap()[kp][0])
                nc.sync.dma_start(a_sb[kp][:, 1], a_t.ap()[kp][1])
            for q in range(2, 4):
                for kp in range(KP):
                    nc.sync.dma_start(a_sb[kp][:, q], a_t.ap()[kp][q])

            if hw_loop:
                rep_ctx = lambda: tc.For_i(0, reps)  # noqa: E731
                n_py_reps = 1
            else:
                rep_ctx = contextlib.nullcontext
                n_py_reps = reps
            def post_pair(big, bc, h, base, pair_i):
                region = big[:, base : base + 2048]
                mxp = opool.tile([128, 2, 32], bf16, tag="mxp", bufs=4)
                if pair_i in DIRECT_PAIRS:
                    # DVE-direct: max-reduce fp32 straight from PSUM (only
                    # one PSUM input allowed per DVE op); split per slot so
                    # each psum slot releases as soon as its half is read;
                    # emits raw max-counts, host thresholds at 15.5
                    rv = region.rearrange("p (c o t) -> p c o t", c=2, t=OR_T)
                    for c in range(2):
                        nc.vector.tensor_reduce(
                            out=mxp[:, c],
                            in_=rv[:, c],
                            axis=mybir.AxisListType.X,
                            op=Alu.max,
                        )
                else:
                    gate = gpool.tile([128, 2048], bf16, tag="gate")
                    # split per 1024-col slot: each starts as soon as its slot's
                    # matmuls finish and releases that slot ~0.9us earlier
                    for c2 in range(2):
                        nc.scalar.activation(
                            out=gate[:, 1024 * c2 : 1024 * (c2 + 1)],
                            in_=region[:, 1024 * c2 : 1024 * (c2 + 1)],
                            func=ActF.Relu, bias=neg15[:],
                        )
                    # 2-level contiguous-half max tree (DVE 2x mode) + reduce
                    gv = gate[:].rearrange("p (c o t) -> p c o t", c=2, t=OR_T)
                    r1 = gpool.tile([128, 2, 32, 16], bf16, tag="r1")
                    nc.vector.tensor_tensor(
                        out=r1[:], in0=gv[:, :, :, 0:16],
                        in1=gv[:, :, :, 16:32], op=Alu.max,
                    )
                    r2 = gpool.tile([128, 2, 32, 8], bf16, tag="r2")
                    nc.vector.tensor_tensor(
                        out=r2[:], in0=r1[:, :, :, 0:8], in1=r1[:, :, :, 8:16],
                        op=Alu.max,
                    )
                    nc.vector.tensor_reduce(
                        out=mxp[:], in_=r2[:],
                        axis=mybir.AxisListType.X, op=Alu.max,
                    )
                # piece covers out rows [128bc, +128), cols [64h, 64h+64)
                nc.sync.dma_start(
                    out_t.ap()[
                        128 * bc : 128 * (bc + 1), 64 * h : 64 * (h + 1)
                    ],
                    mxp[:],
                )

            with rep_ctx():
              for _rep in range(n_py_reps):
                # one big PSUM tile managed as two 2-bank slot-pairs (subtile
                # deps give the PE lookahead).  A piece = (bc, q-half h):
                # 2048 cols of one batch chunk.  kp-outer inside the piece
                # so each lhsT load feeds 4 consecutive streams.
                big = pp.tile([128, 4096], f32, tag="big", bufs=1)
                pair_i = 0
                for h in range(2):
                    for bc in range(8):
                        base = 2048 * (pair_i % 2)
                        for kp in range(KP):
                            for dq in range(2):
                                q = 2 * h + dq
                                for n in range(2):
                                    nc.tensor.matmul(
                                        out=big[:, base + 1024 * dq + 512 * n :
                                                base + 1024 * dq + 512 * (n + 1)],
                                        lhsT=xt_sb[:, 2 * kp : 2 * kp + 2,
                                                   128 * bc : 128 * (bc + 1)],
                                        rhs=a_sb[kp][:, q, :,
                                                     512 * n : 512 * (n + 1)],
                                        start=(kp == 0),
                                        stop=(kp == KP - 1),
                                        perf_mode=DR,
                                    )
                        if stages.startswith("full"):
                            post_pair(big, bc, h, base, pair_i)
                        pair_i += 1
    nc.compile()
    return nc


def _host_inputs(x, weights):
    import ml_dtypes

    f8 = ml_dtypes.float8_e4m3
    ONE = int(np.array(1.0, dtype=f8).view(np.uint8))

    xb = np.asarray(x) != 0  # [B, F] bool
    xt = np.zeros((K, B), np.uint8)
    xt[:F][xb.T] = ONE
    xt[BIAS_ROW, :] = ONE
    # [K, B] -> [KP, 128, 2, B]; row r lives at (kp=r//256, p=r%128, pair=(r//128)%2)
    xt_in = np.ascontiguousarray(
        xt.reshape(KP, 2, 128, B).transpose(0, 2, 1, 3)
    ).view(f8)

    w = np.asarray(weights).astype(np.int64)       # [OUT, OR_T, AND_T]
    allzero = (w == 0).all(axis=-1)                # [OUT, OR_T]
    idx_all = w.reshape(OUT, OR_T * AND_T)
    lut = np.arange(-16, 17, dtype=np.float32).astype(f8).view(np.uint8)

    jj = np.repeat(np.arange(JC), AND_T)           # [JC*AND_T]
    a_maps = []
    for cc in range(N_CORES):
        idx = idx_all[OL * cc : OL * (cc + 1)].reshape(-1)  # [JC*AND_T]
        pos = (idx >= 1) & (idx <= F)
        neg = idx > F
        a_int = np.zeros(K * JC, np.int16)
        np.add.at(a_int, (idx[pos] - 1) * JC + jj[pos], 1)
        np.add.at(a_int, (idx[neg] - 1 - F) * JC + jj[neg], -1)
        a_int = a_int.reshape(K, JC)
        bias = (
            np.bincount(jj[idx == 0], minlength=JC)
            + np.bincount(jj[neg], minlength=JC)
        ).astype(np.int16)
        bias[allzero[OL * cc : OL * (cc + 1)].reshape(-1)] = -16
        a_int[BIAS_ROW, :] = bias
        a_u8 = lut[a_int + 16]                     # [K, JC] e4m3 bytes
        # [K, JC] -> [KP, 4, 128, 2, JC//4]
        a_maps.append(
            np.ascontiguousarray(
                a_u8.reshape(KP, 2, 128, 4, JC // 4).transpose(0, 3, 2, 1, 4)
            ).view(f8)
        )
    return xt_in, a_maps


def _assemble(results):
    import ml_dtypes

    out = np.empty((B, OUT), dtype=bool)
    for cc in range(N_CORES):
        arr = np.ascontiguousarray(results[cc]["out"])
        if arr.dtype != ml_dtypes.bfloat16:
            arr = arr.view(ml_dtypes.bfloat16)
        arr = arr.astype(np.float32)
        blk = np.empty((B, OL), dtype=bool)
        for pair_i in range(16):
            h, bc = divmod(pair_i, 8)
            thr = 15.5 if pair_i in DIRECT_PAIRS else 0.5
            rows = slice(128 * bc, 128 * (bc + 1))
            cols = slice(64 * h, 64 * (h + 1))
            blk[rows, cols] = arr[rows, cols] > thr
        out[:, OL * cc : OL * (cc + 1)] = blk
    return out


def kernel(x, weights):
    from concourse.bass_utils import run_bass_kernel_spmd

    if "nc" not in _cache:
        _cache["nc"] = _build(reps=1)
    nc = _cache["nc"]

    xt_in, a_maps = _host_inputs(x, weights)
    in_maps = [{"xt": xt_in, "a": a_maps[cc]} for cc in range(N_CORES)]
    # transient device/tunnel errors (NRT_EXEC_UNIT_UNRECOVERABLE): retry
    res = None
    for attempt in range(3):
        try:
            res = run_bass_kernel_spmd(
                nc, in_maps, core_ids=list(range(N_CORES))
            )
            break
        except Exception:
            if attempt == 2:
                raise
            import time

            time.sleep(5 * (attempt + 1))
    return _assemble(res.results)
